# revision 29
# baseline (speedup 1.0000x reference)
"""Trainium2 Bass kernel for nn_MetapathRecommender.

Shapes (hardcoded): B=1024, C=8192, P=3, E=64, M=128, H=16, K=8, 8 cores.

Sharding: metapaths (P, C, C) sharded along the last (d) axis -> each core
streams (P, C, C/8) fp8 through the dominant matmul
    pcmT[p] = pc[p] @ metapaths[p][:, dshard]      # (M, D) per p
then computes a partial xT[p] = pcm[p].T @ poolsT locally, ReduceScatters x
per-p (overlapped with later streams), and runs the attention tail +
final cosine head for its own B/8 = 128 batch rows.

Numerics (MODE="e4_dr"): metapaths are streamed as fp8 e4m3 (halves the
dominant DMA vs fp16) and the per-metapath path_card (pc) is kept to ~fp16
accuracy as an fp8 hi+lo pair at a shared 512x scale, so both hi and lo
accumulate into the same PSUM. Both operands being fp8e4 enables DoubleRow
matmuls (2 c-blocks per instruction at 0.5 cyc/row), quartering PE time vs
the fp16 hi+lo baseline. The 1/512 descale is folded into the PSUM->SBUF
copy. End-to-end quantization error (numpy study): l2 ~2.6e-3 vs the 2e-2
gate.

The metapath c-rows are consumed through (128, 4x1024) DMA tiles; the
resulting c-permutation (c = kt*512 + 4*p + j) is folded into the host-side
column order of ceT_aug so the stream lhsT tiles line up.
"""

import sys
from contextlib import ExitStack

sys.path.insert(0, "/opt/trn_rl_repo")

import numpy as np
import ml_dtypes

import concourse.bass as bass
import concourse.tile as tile
from concourse import mybir
from concourse.bass_utils import run_bass_kernel_spmd

F16 = mybir.dt.float16
F32 = mybir.dt.float32
AF = mybir.ActivationFunctionType
ALU = mybir.AluOpType
DR = mybir.MatmulPerfMode.DoubleRow

MODE = "e4_dr"  # "e4_dr" | "e3_single" | "f16_single"

B, C, P, E, M, H, K = 1024, 8192, 3, 64, 128, 16, 8
NCORES = 8
D = C // NCORES          # 1024: d-shard width per core
NB = B // NCORES         # 128: batch rows per core for the tail
NKT2 = C // 512          # 16 tile steps for the big matmul
NCT = C // 128           # 64 card-column blocks
NDT = D // 128           # 8 d tiles
EPS = 1e-12
INV_SQRT_K = 1.0 / float(np.sqrt(np.float32(K)))
PC_SCALE = 512.0

_CACHE = {}


def _mode_dtypes(mode):
    if mode == "e4_dr":
        return mybir.dt.float8e4, ml_dtypes.float8_e4m3
    if mode == "e3_single":
        return mybir.dt.float8e3, ml_dtypes.float8_e3m4
    return F16, np.float16


def _split_multi_waits(nc, cap=1):
    """Walrus in this container only accepts `cap` sync-waits per instruction.

    Move extra waits onto freshly inserted NoOps immediately preceding the
    instruction on the same engine (engines execute their stream in order, so
    waiting on a just-prior NoOp is equivalent)."""
    f = nc.m.functions[0]
    nid = 0
    for blk in f.blocks:
        il = blk.instructions
        i = 0
        while i < len(il):
            inst = il[i]
            si = inst.sync_info
            if si is not None and len(si.on_wait) > cap:
                waits = list(si.on_wait)
                extra, keep = waits[:-cap], waits[-cap:]
                for w in extra:
                    nop = mybir.InstNoOp(
                        name=f"I-wsplit-{nid}", engine=inst.engine,
                        sync_info=mybir.SyncInfo(on_wait=[w], on_update=[]))
                    nid += 1
                    il.insert(i, nop)
                    i += 1
                inst.sync_info = mybir.SyncInfo(
                    on_wait=keep, on_update=list(si.on_update))
            i += 1
    return nid


def build_kernel(no_cc=False, repeat=1, mode=MODE, ablate=None):
    # ablate: None | "no_tail" (skip attention tail) | "dma_only" (metapath
    # stream DMAs only, no compute) | "stream_mm" (DMA + stream matmuls only,
    # pc uninitialized) — timing-attribution variants.
    F8, _ = _mode_dtypes(mode)
    nc = bass.Bass(num_devices=NCORES)

    # ---- kernel I/O (per-core shards / replicated small tensors) ----
    mp_d = nc.dram_tensor("mp_shard", [P, C, D], F8, kind="ExternalInput")
    poolsT_d = nc.dram_tensor("poolsT_shard", [D, B], F16, kind="ExternalInput")
    ceT_aug_d = nc.dram_tensor("ceT_aug", [E + 1, C], F16, kind="ExternalInput")
    mpk_aug_d = nc.dram_tensor("mpk_aug", [E + 1, P * M], F16, kind="ExternalInput")
    ncT_d = nc.dram_tensor("ncT", [E, C], F16, kind="ExternalInput")
    wqkv_d = nc.dram_tensor("wqkv", [M, 3 * H * K], F32, kind="ExternalInput")
    wo_d = nc.dram_tensor("wo", [H * K, M], F32, kind="ExternalInput")
    bqkv_d = nc.dram_tensor("bqkv_bc", [NB, 3 * H * K], F32, kind="ExternalInput")
    bo3_d = nc.dram_tensor("bo3_col", [M, 1], F32, kind="ExternalInput")
    pk_d = nc.dram_tensor("pool_kernel", [M, E], F32, kind="ExternalInput")
    pb_d = nc.dram_tensor("pool_bias_bc", [NB, E], F32, kind="ExternalInput")
    ident_h_d = nc.dram_tensor("ident_h", [128, 128], F16, kind="ExternalInput")
    ident_f_d = nc.dram_tensor("ident_f", [128, 128], F32, kind="ExternalInput")

    out_d = nc.dram_tensor("out", [NB, C], F16, kind="ExternalOutput")

    with ExitStack() as ctx:
        tc = ctx.enter_context(tile.TileContext(nc, num_cores=NCORES))

        const = ctx.enter_context(tc.tile_pool(name="const", bufs=1))
        dram = ctx.enter_context(tc.tile_pool(name="dram", bufs=1, space="DRAM"))

        # ---------- load constants / replicated weights ----------
        poolsT_sb = const.tile([128, NDT, B], F16)  # (d%128, dtile, b)
        nc.scalar.dma_start(poolsT_sb[:], poolsT_d.ap().rearrange("(t p) b -> p t b", p=128))
        mpk_aug_sb = const.tile([E + 1, P * M], F16)
        nc.sync.dma_start(mpk_aug_sb[:], mpk_aug_d[:, :])
        ncT_sb = const.tile([E, C], F16)
        nc.scalar.dma_start(ncT_sb[:], ncT_d[:, :])
        wqkv_sb = const.tile([M, 3 * H * K], F32)
        nc.scalar.dma_start(wqkv_sb[:], wqkv_d[:, :])
        wo_sb = const.tile([H * K, M], F32)
        nc.scalar.dma_start(wo_sb[:], wo_d[:, :])
        bqkv_sb = const.tile([NB, 3 * H * K], F32)
        nc.scalar.dma_start(bqkv_sb[:], bqkv_d[:, :])
        bo3_sb = const.tile([M, 1], F32)
        nc.scalar.dma_start(bo3_sb[:], bo3_d[:, :])
        pk_sb = const.tile([M, E], F32)
        nc.scalar.dma_start(pk_sb[:], pk_d[:, :])
        pb_sb = const.tile([NB, E], F32)
        nc.scalar.dma_start(pb_sb[:], pb_d[:, :])
        ident_h = const.tile([128, 128], F16)
        nc.scalar.dma_start(ident_h[:], ident_h_d[:, :])
        ident_f = const.tile([128, 128], F32)
        nc.scalar.dma_start(ident_f[:], ident_f_d[:, :])
        half_sb = const.tile([128, 1], F32)
        nc.vector.memset(half_sb[:], 0.5)

        for _rep in range(repeat):
            # stream-phase psum pools (released before the tail)
            stream_psum_ctx = ExitStack()
            small_psum = stream_psum_ctx.enter_context(
                tc.tile_pool(name="small_psum", bufs=2, space="PSUM"))   # 2 banks
            mm_psum = stream_psum_ctx.enter_context(
                tc.tile_pool(name="mm_psum", bufs=2, space="PSUM"))      # 4 banks
            trx_psum = stream_psum_ctx.enter_context(
                tc.tile_pool(name="trx_psum", bufs=1, space="PSUM"))     # 2 banks
            stream_sbuf_ctx = ExitStack()
            mp_pool = stream_sbuf_ctx.enter_context(tc.tile_pool(name="mp", bufs=6))
            pcm_pool = stream_sbuf_ctx.enter_context(tc.tile_pool(name="pcm", bufs=2))
            ncn_pool = stream_sbuf_ctx.enter_context(tc.tile_pool(name="ncn", bufs=3))
            ceh_pool = stream_sbuf_ctx.enter_context(tc.tile_pool(name="ceh", bufs=1))

            # ---------- phase B: pc = silu(ceT_aug.T @ mpk_aug) -> fp8 -------
            # One fp16 matmul per c-block covers all P metapaths (N = P*M).
            # Bias is folded in via the augmented ones-row of ceT_aug.
            # pc is stored as PC_SCALE*pc in fp8 hi + lo at the SAME scale, so
            # both accumulate into one PSUM; descale happens at PSUM readout.
            pcT_hi = const.tile([128, NCT, P, M], F8, name="pcT_hi")
            if mode == "e4_dr":
                pcT_lo = const.tile([128, NCT, P, M], F8, name="pcT_lo")
            if ablate == "stream_mm":
                nc.vector.memset(pcT_hi[:], 0)
                if mode == "e4_dr":
                    nc.vector.memset(pcT_lo[:], 0)
            for half in range(0 if ablate in ("dma_only", "stream_mm") else 2):
                ceh = ceh_pool.tile([E + 1, C // 2], F16, tag="ceh")
                nc.sync.dma_start(ceh[:], ceT_aug_d[:, half * (C // 2):(half + 1) * (C // 2)])
                for cb0 in range(NCT // 2):
                    cb = half * (NCT // 2) + cb0
                    ps = small_psum.tile([128, P * M], F32, tag="sm")
                    nc.tensor.matmul(
                        ps[:], lhsT=ceh[:, cb0 * 128:(cb0 + 1) * 128],
                        rhs=mpk_aug_sb[:], start=True, stop=True)
                    # fp16 intermediate: DVE reads run at 2x; the fp16
                    # rounding of pc (~5e-4 rel) is negligible vs fp8's 3%.
                    tmp = ncn_pool.tile([128, P * M], F16, tag="pctmp")
                    nc.scalar.activation(tmp[:], ps[:], AF.Silu)
                    hi_v = pcT_hi[:, cb, :, :].rearrange("q p m -> q (p m)")
                    nc.vector.tensor_scalar_mul(hi_v, tmp[:], PC_SCALE)
                    if mode == "e4_dr":
                        lo_v = pcT_lo[:, cb, :, :].rearrange("q p m -> q (p m)")
                        nc.vector.scalar_tensor_tensor(
                            lo_v, tmp[:], PC_SCALE, hi_v, ALU.mult, ALU.subtract)

            # ---------- phase C: the big stream + partial x + per-p RS -------
            # p0 and p1 are interleaved tile-by-tile (two live PSUM
            # accumulators) so the pc-production rate of phase B keeps ahead
            # of consumption; p2 then streams alone at DMA rate while the
            # p0/p1 drains (transpose + x matmul + RS) hide under it.
            x_sb = [const.tile([M, B], F32, name=f"x{p}") for p in range(P)]
            cc_in = [dram.tile([NCORES, M, NB], F32, name=f"cci{p}") for p in range(P)]
            cc_out = [dram.tile([M, NB], F32, name=f"cco{p}") for p in range(P)]

            def stream_tile(p, kt, ps):
                mp_t = mp_pool.tile([128, 4, D], F8, tag="mpt")
                nc.sync.dma_start(
                    mp_t[:],
                    mp_d[p, kt * 512:(kt + 1) * 512, :].rearrange(
                        "(q j) d -> q j d", j=4))
                if ablate == "dma_only":
                    return
                if mode == "e4_dr":
                    # both column halves run back-to-back under the same
                    # stationary operand so the PE can skip weight reloads
                    for j0 in (0, 2):
                        cb0 = kt * 4 + j0
                        first = (kt == 0 and j0 == 0)
                        last = (kt == NKT2 - 1 and j0 == 2)
                        for lhsT, st, sp in ((pcT_hi, first, False),
                                             (pcT_lo, False, last)):
                            for ch in range(2):
                                nc.tensor.matmul(
                                    ps[:, ch * 512:(ch + 1) * 512],
                                    lhsT=lhsT[:, cb0:cb0 + 2, p, :],
                                    rhs=mp_t[:, j0:j0 + 2, ch * 512:(ch + 1) * 512],
                                    start=st, stop=sp, perf_mode=DR)
                else:
                    for j in range(4):
                        cb = kt * 4 + j
                        first = (kt == 0 and j == 0)
                        last = (kt == NKT2 - 1 and j == 3)
                        for ch in range(2):
                            rhs = mp_t[:, j, ch * 512:(ch + 1) * 512]
                            nc.tensor.matmul(
                                ps[:, ch * 512:(ch + 1) * 512],
                                lhsT=pcT_hi[:, cb, p, :], rhs=rhs,
                                start=first, stop=last)

            pcmT_sb = {}

            def drain_copy(p, ps):
                # pcmT (m part, d free) descale f32 -> fp16 sbuf
                pcmT = pcm_pool.tile([128, D], F16, tag="pcmT")
                nc.scalar.activation(pcmT[:, 0:512], ps[:, 0:512], AF.Copy,
                                     scale=1.0 / PC_SCALE)
                nc.vector.tensor_scalar_mul(pcmT[:, 512:1024], ps[:, 512:1024],
                                            1.0 / PC_SCALE)
                pcmT_sb[p] = pcmT

            def drain_rest(p):
                pcmT = pcmT_sb[p]
                # transpose to (d part, m free)
                trp = trx_psum.tile([128, D], F16, tag="trx")
                for dt in range(NDT):
                    nc.tensor.transpose(trp[:, dt * 128:(dt + 1) * 128],
                                        pcmT[:, dt * 128:(dt + 1) * 128], ident_h[:])
                pcm_d = pcm_pool.tile([128, D], F16, tag="pcmd")
                nc.vector.tensor_copy(pcm_d[:, 0:512], trp[:, 0:512])
                nc.scalar.activation(pcm_d[:, 512:1024], trp[:, 512:1024], AF.Copy)
                # xT[p] (m part, b free) partial = sum_dt pcm_d[dt].T @ poolsT[dt]
                xps = trx_psum.tile([128, B], F32, tag="trx")
                for half in range(2):
                    for dt in range(NDT):
                        nc.tensor.matmul(
                            xps[:, half * 512:(half + 1) * 512],
                            lhsT=pcm_d[:, dt * 128:(dt + 1) * 128],
                            rhs=poolsT_sb[:, dt, half * 512:(half + 1) * 512],
                            start=(dt == 0), stop=(dt == NDT - 1),
                        )
                nc.vector.tensor_copy(x_sb[p][:, 0:512], xps[:, 0:512])
                nc.scalar.activation(x_sb[p][:, 512:1024], xps[:, 512:1024], AF.Copy)
                # reduce-scatter this p's x right away (overlaps later work):
                # cc_in[p] block j = partial xT[p] for rank j's batch rows.
                nc.sync.dma_start(
                    cc_in[p].rearrange("j m b -> m j b"),
                    x_sb[p][:].rearrange("m (j b) -> m j b", j=NCORES))
                if no_cc:
                    nc.sync.dma_start(cc_out[p][:, :], cc_in[p][0, :, :])
                else:
                    nc.gpsimd.collective_compute(
                        "ReduceScatter", ALU.add,
                        replica_groups=[list(range(NCORES))],
                        ins=[cc_in[p][:, :, :]],
                        outs=[cc_out[p][:, :]],
                    )

            ps0 = mm_psum.tile([128, D], F32, tag="mmps")
            ps1 = mm_psum.tile([128, D], F32, tag="mmps")
            for kt in range(NKT2):
                stream_tile(0, kt, ps0)
                stream_tile(1, kt, ps1)
            if ablate in ("dma_only", "stream_mm"):
                if ablate == "stream_mm":
                    ps2 = mm_psum.tile([128, D], F32, tag="mmps")
                else:
                    ps2 = ps0
                for kt in range(NKT2):
                    stream_tile(2, kt, ps2)
                stream_psum_ctx.close()
                stream_sbuf_ctx.close()
                continue
            drain_copy(0, ps0)
            drain_copy(1, ps1)
            # ps2 reuses ps0's buffer; allocated after drain_copy(0) is emitted
            # so the WAR dependency on the copy is tracked. The PE-heavy drain
            # remainders are interleaved into p2's DMA-bound stream.
            ps2 = mm_psum.tile([128, D], F32, tag="mmps")
            for kt in range(NKT2):
                stream_tile(2, kt, ps2)
                if kt == 0:
                    drain_rest(0)
                elif kt == 4:
                    drain_rest(1)
            drain_copy(2, ps2)
            drain_rest(2)

            # release stream-phase psum pools so the tail can use the banks
            stream_psum_ctx.close()
            stream_sbuf_ctx.close()
            if ablate == "no_tail":
                continue

            # ---------- tail: attention on this core's batch shard ----------
            tail_ctx = ExitStack()
            tail = tail_ctx.enter_context(tc.tile_pool(name="tail", bufs=1))
            tail_ps = tail_ctx.enter_context(
                tc.tile_pool(name="tail_ps", bufs=1, space="PSUM"))

            xt = [tail.tile([M, NB], F32, name=f"xt{p}") for p in range(P)]
            for p in range(P):
                nc.sync.dma_start(xt[p][:], cc_out[p][:, :])

            # fused QKV (1/sqrt(K) folded into Wq host-side): one matmul + one
            # bias-add per p, into a single (b, p, {q,k,v}, h, k) tile so the
            # attention below runs as a handful of wide strided DVE ops
            # instead of ~50 small serialized ones.
            qkv = tail.tile([NB, P, 3, H, K], F32)
            for p in range(P):
                qp = tail_ps.tile([NB, 3 * H * K], F32, tag="qkv", bufs=2)
                nc.tensor.matmul(qp[:], lhsT=xt[p][:], rhs=wqkv_sb[:],
                                 start=True, stop=True)
                nc.vector.tensor_add(
                    qkv[:, p, :, :, :].rearrange("b t h k -> b (t h k)"),
                    qp[:], bqkv_sb[:])

            # scores s_all (b, p, h, q2) = sum_k q[b,p,h,k] * k[b,q2,h,k]
            k_hat = qkv[:, :, 1, :, :].rearrange("b q h k -> b h q k")
            s_all = tail.tile([NB, P, H, P], F32)
            for p in range(P):
                q_v = qkv[:, p, 0, :, :][:, :, None, :]          # b h 1 k
                a_b, k_b = bass.broadcast_tensor_aps(q_v, k_hat)
                tmp = tail.tile([NB, H, P, K], F32, tag="stmp", bufs=2)
                nc.vector.tensor_tensor(tmp[:], a_b, k_b, ALU.mult)
                nc.vector.tensor_reduce(
                    s_all[:, p, :, :], tmp[:], axis=mybir.AxisListType.X, op=ALU.add)

            # softmax over q2 (innermost), batched over (p, h)
            mx = tail.tile([NB, P, H], F32)
            nc.vector.tensor_reduce(mx[:], s_all[:], axis=mybir.AxisListType.X,
                                    op=ALU.max)
            e_all = tail.tile([NB, P, H, P], F32)
            m_b, s_b = bass.broadcast_tensor_aps(mx[:, :, :, None], s_all[:])
            nc.vector.tensor_tensor(e_all[:], s_b, m_b, ALU.subtract)
            nc.scalar.activation(e_all[:], e_all[:], AF.Exp)
            den = tail.tile([NB, P, H], F32)
            nc.vector.tensor_reduce(den[:], e_all[:], axis=mybir.AxisListType.X,
                                    op=ALU.add)
            nc.vector.reciprocal(den[:], den[:])
            d_b, e_b = bass.broadcast_tensor_aps(den[:, :, :, None], e_all[:])
            nc.vector.tensor_tensor(e_all[:], e_b, d_b, ALU.mult)

            # o[p] (b, h, k) = sum_q2 attn * v; transpose; project with Wo
            v_hat = qkv[:, :, 2, :, :].rearrange("b q h k -> b h k q")
            att_ps = tail_ps.tile([M, NB], F32, tag="attps")
            for p in range(P):
                a_v = e_all[:, p, :, :][:, :, None, :]           # b h 1 q2
                a_b, v_b = bass.broadcast_tensor_aps(a_v, v_hat)
                tmp6 = tail.tile([NB, H, K, P], F32, tag="tmp6", bufs=2)
                nc.vector.tensor_tensor(tmp6[:], a_b, v_b, ALU.mult)
                o_t = tail.tile([NB, H * K], F32, tag="o_t", bufs=2)
                nc.vector.tensor_reduce(
                    o_t[:].rearrange("b (h k) -> b h k", k=K), tmp6[:],
                    axis=mybir.AxisListType.X, op=ALU.add)
                oT_ps = tail_ps.tile([NB, H * K], F32, tag="oTps", bufs=1)
                nc.tensor.transpose(oT_ps[:], o_t[:], ident_f[:])
                oT = tail.tile([H * K, NB], F32, tag="oT", bufs=2)
                nc.vector.tensor_copy(oT[:], oT_ps[:])
                nc.tensor.matmul(att_ps[:], lhsT=wo_sb[:], rhs=oT[:],
                                 start=(p == 0), stop=(p == P - 1))

            # preT (m, b) = attendedT summed over p, + 3*bo
            preT = tail.tile([M, NB], F32)
            nc.scalar.activation(preT[:], att_ps[:], AF.Identity, bias=bo3_sb[:])

            # pool_embeds (b, e) = preT.T @ pool_kernel + pool_bias
            pe_ps = tail_ps.tile([NB, E], F32, tag="pe")
            nc.tensor.matmul(pe_ps[:], lhsT=preT[:], rhs=pk_sb[:], start=True, stop=True)
            pe = tail.tile([NB, E], F32)
            nc.vector.tensor_add(pe[:], pe_ps[:], pb_sb[:])
            # l2 normalize along e
            sq2 = tail.tile([NB, E], F32)
            ss2 = tail.tile([NB, 1], F32)
            nc.scalar.activation(sq2[:], pe[:], AF.Square, accum_out=ss2[:])
            nc.vector.tensor_scalar_max(ss2[:], ss2[:], EPS)
            nc.scalar.sqrt(ss2[:], ss2[:])
            rr = tail.tile([NB, 1], F32)
            nc.vector.reciprocal(rr[:], ss2[:])
            npn = tail.tile([NB, E], F32)
            nc.scalar.activation(npn[:], pe[:], AF.Copy, scale=rr[:])
            # transpose to (e, b), cast fp16
            npT_ps = tail_ps.tile([E, NB], F32, tag="npT")
            nc.tensor.transpose(npT_ps[:], npn[:], ident_f[:])
            npT = tail.tile([E, NB], F16)
            nc.vector.tensor_copy(npT[:], npT_ps[:])

            # final: out (b, c) = (npT.T @ ncT + 1) / 2, streamed out per chunk
            # (scale/shift alternates between act and vector engines so the
            # 16-chunk pipeline is not serialized on one engine)
            out_sb = tail.tile([NB, C], F16)
            for ch in range(C // 512):
                fp = tail_ps.tile([NB, 512], F32, tag="fin", bufs=2)
                nc.tensor.matmul(fp[:], lhsT=npT[:], rhs=ncT_sb[:, ch * 512:(ch + 1) * 512],
                                 start=True, stop=True)
                o_v = out_sb[:, ch * 512:(ch + 1) * 512]
                if ch % 2 == 0:
                    nc.scalar.activation(o_v, fp[:], AF.Identity,
                                         bias=half_sb[:NB, :], scale=0.5)
                else:
                    h_b, f_b = bass.broadcast_tensor_aps(half_sb[:NB, :], fp[:])
                    nc.vector.scalar_tensor_tensor(
                        o_v, f_b, 0.5, h_b, ALU.mult, ALU.add)
                nc.sync.dma_start(out_d[:, ch * 512:(ch + 1) * 512], o_v)
            tail_ctx.close()

    _split_multi_waits(nc)
    return nc


def _prep_inputs(inputs, mode=MODE):
    h16 = np.float16
    _, f8np = _mode_dtypes(mode)
    pools = np.asarray(inputs["pools"], np.float32)
    metapaths = np.asarray(inputs["metapaths"], np.float32)
    ce = np.asarray(inputs["card_embeddings"], np.float32)
    mpk = np.asarray(inputs["mp_kernels"], np.float32)
    mpb = np.asarray(inputs["mp_biases"], np.float32)

    mp_h = metapaths.astype(f8np)
    poolsT = np.ascontiguousarray(pools.T).astype(h16)
    ceT_aug = np.concatenate([ce.T, np.ones((1, C), np.float32)], axis=0)
    # column permutation matching the (128, 4x1024) metapath stream tiles:
    # block cb = kt*4 + j holds columns c = kt*512 + 4*p + j for p in 0..127
    idx = np.arange(C).reshape(NKT2, 128, 4).transpose(0, 2, 1).reshape(-1)
    ceT_aug = np.ascontiguousarray(ceT_aug[:, idx]).astype(h16)
    mpk_aug = np.concatenate([mpk, mpb.transpose(0, 2, 1)], axis=1)  # (P, E+1, M)
    mpk_aug = np.ascontiguousarray(
        mpk_aug.transpose(1, 0, 2).reshape(E + 1, P * M)).astype(h16)

    # normalized cards (host): matches tf.l2_normalize(ce, axis=1)
    nrm = np.sqrt(np.maximum((ce * ce).sum(axis=1, keepdims=True), EPS))
    ncT = np.ascontiguousarray((ce / nrm).T).astype(h16)
    wq = np.asarray(inputs["Wq"], np.float32).reshape(M, H * K)
    wk = np.asarray(inputs["Wk"], np.float32).reshape(M, H * K)
    wv = np.asarray(inputs["Wv"], np.float32).reshape(M, H * K)
    bq = np.asarray(inputs["bq"], np.float32).reshape(H * K)
    bk = np.asarray(inputs["bk"], np.float32).reshape(H * K)
    bv = np.asarray(inputs["bv"], np.float32).reshape(H * K)
    wqkv = np.ascontiguousarray(
        np.concatenate([wq * INV_SQRT_K, wk, wv], axis=1))
    bqkv = np.concatenate([bq * INV_SQRT_K, bk, bv])

    com = {
        "ceT_aug": ceT_aug,
        "mpk_aug": mpk_aug,
        "ncT": ncT,
        "wqkv": wqkv,
        "bqkv_bc": np.ascontiguousarray(np.broadcast_to(
            bqkv.reshape(1, 3 * H * K), (NB, 3 * H * K))),
        "wo": np.ascontiguousarray(np.asarray(inputs["Wo"], np.float32).reshape(H * K, M)),
        "bo3_col": np.ascontiguousarray(
            (P * np.asarray(inputs["bo"], np.float32)).reshape(M, 1)),
        "pool_kernel": np.ascontiguousarray(np.asarray(inputs["pool_kernel"], np.float32)),
        "pool_bias_bc": np.ascontiguousarray(np.broadcast_to(
            np.asarray(inputs["pool_bias"], np.float32).reshape(1, E), (NB, E))),
        "ident_h": np.eye(128, dtype=h16),
        "ident_f": np.eye(128, dtype=np.float32),
    }
    in_maps = []
    for i in range(NCORES):
        m = dict(com)
        m["mp_shard"] = np.ascontiguousarray(mp_h[:, :, i * D:(i + 1) * D])
        m["poolsT_shard"] = np.ascontiguousarray(poolsT[i * D:(i + 1) * D, :])
        in_maps.append(m)
    return in_maps


def kernel(**inputs) -> np.ndarray:
    if "nc" not in _CACHE:
        _CACHE["nc"] = build_kernel()
    nc = _CACHE["nc"]
    in_maps = _prep_inputs(inputs)
    res = run_bass_kernel_spmd(nc, in_maps, core_ids=list(range(NCORES)))
    outs = [np.asarray(res.results[i]["out"]).astype(np.float32)
            for i in range(NCORES)]
    return np.concatenate(outs, axis=0)


if __name__ == "__main__":
    nc = build_kernel()
    print("kernel built OK")
